# revision 26
# baseline (speedup 1.0000x reference)
"""CapsLayer2D dynamic-routing kernel for 8 Trainium2 NeuronCores.

Full inputs:  inputs [32,14,14,32,8] f32, W [16,32,8,16] f32
Full output:  out [32,14,14,16,16] f32

Sharding: pure data parallel over batch (4 batches / core -> 784 routing
locations per core). W replicated.

v4: p2 stored in (k,j,n) layout so the weighted-sum multiply reads c
directly through a middle-broadcast AP at DVE 2x rate (no scalar cj
expansion); the agreement p2.out contraction is refactored through the
tensor engine as g[l,k,n,i] = sum_j W[k,n,i,j] out[l,k,j] (matmuls
against transposed out), leaving only a 4096-wide multiply + i-tree on
the DVE; reduction trees keep the reduced dim innermost so every stage
runs packed fp16 at 2x; output is DMA'd fp16 and widened on host.
Tiles are processed in pairs with iteration-level interleave so each
tile's PE/scalar g-phase hides under the other tile's DVE phase.
"""

import sys

sys.path.insert(0, "/opt/trn_rl_repo")

import numpy as np

import concourse.bass as bass
import concourse.mybir as mybir
from concourse.bacc import Bacc
from concourse.tile import TileContext

F32 = mybir.dt.float32
F16 = mybir.dt.float16
ADD = mybir.AluOpType.add
MULT = mybir.AluOpType.mult
DIV = mybir.AluOpType.divide
AX = mybir.AxisListType.X
EXP = mybir.ActivationFunctionType.Exp
SQRT = mybir.ActivationFunctionType.Sqrt
SQUARE = mybir.ActivationFunctionType.Square

EPS = 1e-7
B, R, C, N, I = 32, 14, 14, 32, 8
K, J = 16, 16
NCORES = 8
BC = B // NCORES            # batches per core
L = BC * R * C              # 784 locations per core
PT = 112                    # locations per partition-tile
NT = L // PT                # 7 tiles
NI = N * I                  # 256
KJ = K * J                  # 256
JN = J * N                  # 512 (= k-stride in p2 (k,j,n))
KN = K * N                  # 512
KNI = K * N * I             # 4096
KJN = K * J * N             # 8192


def _ap(base, dims):
    """AP over tile `base` ([part, free] contiguous) with free dims
    [(step,count)...] in elements; step 0 = broadcast."""
    return bass.AP(base.tensor, base.offset, [list(base.ap[0])] + [list(d) for d in dims])


def _apo(base, off, dims):
    """Same, with an element offset into the free dim."""
    return bass.AP(base.tensor, base.offset + off,
                   [list(base.ap[0])] + [list(d) for d in dims])


def build_bass():
    nc = Bacc()
    x_d = nc.declare_dram_parameter("x", [L, NI], F16, isOutput=False)
    wsum_d = nc.declare_dram_parameter("wsum", [2, 128, KJ], F16, isOutput=False)
    wbd_d = nc.declare_dram_parameter("wbd", [2, 128, KJN // 2], F16, isOutput=False)
    wg_d = nc.declare_dram_parameter("wg", [2, 128, KNI // 2], F16, isOutput=False)
    eye_d = nc.declare_dram_parameter("eye", [128, 128], F16, isOutput=False)
    out_d = nc.declare_dram_parameter("out", [L, KJ], F16, isOutput=True)

    with TileContext(nc) as tc:
        import contextlib
        ctx = contextlib.ExitStack()
        with ctx:
            cpool = ctx.enter_context(tc.tile_pool(name="const", bufs=1))
            wpool = ctx.enter_context(tc.tile_pool(name="work", bufs=1))
            bigpool = ctx.enter_context(tc.tile_pool(name="big", bufs=1))
            tpool = ctx.enter_context(tc.tile_pool(name="tree", bufs=1))
            pspool = ctx.enter_context(tc.tile_pool(name="ps", bufs=1, space="PSUM"))
            psmm = ctx.enter_context(tc.tile_pool(name="psmm", bufs=2, space="PSUM"))
            psg = ctx.enter_context(tc.tile_pool(name="psg", bufs=2, space="PSUM"))

            wsum0 = cpool.tile([128, KJ], F16)
            wsum1 = cpool.tile([128, KJ], F16)
            wbd0 = cpool.tile([128, KJN // 2], F16)
            wbd1 = cpool.tile([128, KJN // 2], F16)
            wg0 = cpool.tile([128, KNI // 2], F16)
            wg1 = cpool.tile([128, KNI // 2], F16)
            eye = cpool.tile([128, 128], F16)
            nc.sync.dma_start(eye[:], eye_d[:])

            # PE warm-up: absorb each const DMA tick into PE's vector clock
            # one at a time, so no later LDWEIGHTS needs >1 sync wait
            # (HW limit: one wait slot on LDW).
            def warm(*wts):
                for wt in wts:
                    ps_w = psmm.tile([112, 512], F32, tag="mm", name="ps_w")
                    nc.tensor.matmul(ps_w[:, :KJ], wt[:, :112], wt[:, :KJ],
                                     start=True, stop=True)
            ps_wt = pspool.tile([128, 128], F16, tag="psT", name="ps_wt")
            nc.tensor.transpose(ps_wt[:], eye[:], eye[:])

            def squash(s_sb, out_sb, tag):
                """out = squash(s) over j; s_sb [PT,KJ] f32, out_sb [PT,KJ]
                f16 ((k,j) layout). f = sq/((1+sq)*sqrt(sq+eps)); out = s*f."""
                ssq = wpool.tile([PT, KJ], F16, tag=f"ssq{tag}", name=f"ssq{tag}")
                sq = wpool.tile([PT, K], F32, tag=f"sq{tag}", name=f"sq{tag}")
                rti = wpool.tile([PT, K], F32, tag=f"rti{tag}", name=f"rti{tag}")
                rt = wpool.tile([PT, K], F32, tag=f"rt{tag}", name=f"rt{tag}")
                den = wpool.tile([PT, K], F32, tag=f"den{tag}", name=f"den{tag}")
                rden = wpool.tile([PT, K], F32, tag=f"rd{tag}", name=f"rd{tag}")
                f = wpool.tile([PT, K], F32, tag=f"f{tag}", name=f"f{tag}")
                nc.scalar.activation(ssq[:], s_sb[:], SQUARE)
                nc.vector.tensor_reduce(
                    sq[:], _ap(ssq, [[J, K], [1, J]]), AX, ADD)
                nc.vector.tensor_scalar(rti[:], sq[:], EPS, None, ADD)
                nc.scalar.activation(rt[:], rti[:], SQRT)
                # den = (sq + 1) * rt, fused on DVE
                nc.vector.scalar_tensor_tensor(
                    den[:], sq[:], 1.0, rt[:], ADD, MULT)
                nc.vector.reciprocal(rden[:], den[:])
                nc.vector.tensor_tensor(f[:], sq[:], rden[:], MULT)
                nc.vector.tensor_tensor(
                    _ap(out_sb, [[J, K], [1, J]]),
                    _ap(s_sb, [[J, K], [1, J]]),
                    _ap(f, [[1, K], [0, J]]),
                    MULT)

            def stage_a1x(t, sfx):
                """x DMA + transposes. Returns state dict."""
                st = {"t": t, "sfx": sfx}
                x_sb = wpool.tile([PT, NI], F16, tag=f"x{sfx}", name="x_sb")
                nc.sync.dma_start(x_sb[:], x_d[t * PT:(t + 1) * PT, :])
                st["x"] = x_sb
                xt = []
                for h in range(2):
                    ps_t = pspool.tile([128, PT], F16, tag="psT", name="ps_t")
                    xth = wpool.tile([128, PT], F16, tag=f"xT{h}{sfx}",
                                     name=f"xT{h}")
                    nc.tensor.transpose(
                        ps_t[:], x_sb[:, h * 128:(h + 1) * 128], eye[:PT, :PT])
                    nc.scalar.copy(xth[:], ps_t[:])
                    xt.append(xth)
                st["xt"] = xt
                return st

            def stage_a1p(st, lo=0, hi=16):
                """p2 matmuls + psum->sbuf copies; p2 layout (k,j,n).
                chunk ch covers n-pair (2ch,2ch+1); within-chunk psum cols
                (k,j,d) so the copy is dst-inner [1,2]."""
                sfx = st["sfx"]
                if lo == 0:
                    p2 = bigpool.tile([PT, KJN], F16, tag=f"p2{sfx}",
                                      name="p2")
                    st["p2"] = p2
                p2 = st["p2"]
                for ch in range(lo, hi):
                    h = ch // 8
                    wb = (wbd0, wbd1)[h]
                    ps = psmm.tile([PT, 512], F32, tag="mm", name="ps_mm")
                    nc.tensor.matmul(
                        ps[:], st["xt"][h][:],
                        wb[:, (ch % 8) * 512:(ch % 8 + 1) * 512],
                        start=True, stop=True)
                    dst = _apo(p2, 2 * ch, [[JN, K], [N, J], [1, 2]])
                    src = _ap(ps, [[32, K], [2, J], [1, 2]])
                    nc.scalar.copy(dst, src)

            def stage_a2(st):
                """Iteration 1 (uniform c): s = mean_n p2, squash."""
                xt = st["xt"]
                sfx = st["sfx"]
                ps_s = pspool.tile([PT, KJ], F32, tag="s", name="ps_s")
                nc.tensor.matmul(ps_s[:], xt[0][:], wsum0[:], start=True, stop=False)
                nc.tensor.matmul(ps_s[:], xt[1][:], wsum1[:], start=False, stop=True)
                out_h = wpool.tile([PT, KJ], F16, tag=f"oh{sfx}", name="out_h")
                squash(ps_s, out_h, f"a{sfx}")
                st["out"] = out_h

            def stage_g(st, it):
                """PE/scalar phase: outT transposes + g matmuls + copies.
                g[l,(k,n,i)] = sum_j W[k,n,i,j] * out[l,k,j], fp16 SBUF."""
                sfx = st["sfx"]
                out_h = st["out"]
                g_sb = tpool.tile([PT, KNI], F16, tag=f"g{sfx % 3}",
                                  name="g_sb")
                for h in range(2):
                    ps_t = pspool.tile([128, PT], F16, tag="psT", name="ps_ot")
                    oth = wpool.tile([128, PT], F16, tag=f"oT{h}{sfx}",
                                     name=f"oT{h}")
                    nc.tensor.transpose(
                        ps_t[:], out_h[:, h * 128:(h + 1) * 128],
                        eye[:PT, :PT])
                    nc.scalar.copy(oth[:], ps_t[:])
                    wgh = (wg0, wg1)[h]
                    for piece in range(4):
                        g_ps = psg.tile([PT, 512], F32, tag="gps",
                                        name="g_ps")
                        nc.tensor.matmul(
                            g_ps[:], oth[:, :PT],
                            wgh[:, piece * 512:(piece + 1) * 512],
                            start=True, stop=True)
                        nc.scalar.copy(
                            g_sb[:, (h * 4 + piece) * 512:
                                 (h * 4 + piece + 1) * 512], g_ps[:])
                st["g"] = g_sb

            def stage_v(st, it):
                """DVE phase: agreement (x*g + i-tree) -> b; softmax -> c;
                weighted sum (p2*c_bcast + n-tree) -> s2; squash -> out."""
                sfx = st["sfx"]
                p2, g_sb, x_sb = st["p2"], st["g"], st["x"]
                # ---- agreement: tmp_a[l,(k,n,i)] = x[l,(n,i)] * g ----
                tmp_a = tpool.tile([PT, KNI], F16, tag="ta", name="tmp_a")
                nc.vector.tensor_tensor(
                    tmp_a[:], g_sb[:],
                    _ap(x_sb, [[0, K], [I, N], [1, I]]),
                    MULT)
                a1 = tpool.tile([PT, KNI // 2], F16, tag="a1", name="a1")
                nc.vector.tensor_tensor(
                    _ap(a1, [[4, KN], [1, 4]]),
                    _ap(tmp_a, [[8, KN], [1, 4]]),
                    _apo(tmp_a, 4, [[8, KN], [1, 4]]),
                    ADD)
                a2 = tpool.tile([PT, KNI // 4], F16, tag="a2", name="a2")
                nc.vector.tensor_tensor(
                    _ap(a2, [[2, KN], [1, 2]]),
                    _ap(a1, [[4, KN], [1, 2]]),
                    _apo(a1, 2, [[4, KN], [1, 2]]),
                    ADD)
                if it == 0:
                    b_sb = wpool.tile([PT, KN], F32, tag=f"b{sfx}", name="b_sb")
                    nc.vector.tensor_tensor(
                        b_sb[:],
                        _ap(a2, [[2, KN]]),
                        _apo(a2, 1, [[2, KN]]),
                        ADD)
                    st["b"] = b_sb
                else:
                    a3 = tpool.tile([PT, KNI // 8], F16, tag="a3",
                                    name="a3")
                    nc.vector.tensor_tensor(
                        a3[:],
                        _ap(a2, [[2, KN]]),
                        _apo(a2, 1, [[2, KN]]),
                        ADD)
                    b_sb = st["b"]
                    nc.vector.tensor_tensor(b_sb[:], b_sb[:], a3[:], ADD)

                # ---- softmax over n (b bounded; no max-sub), c = e*r ----
                e_sb = wpool.tile([PT, KN], F32, tag=f"e{sfx}", name="e_sb")
                nc.scalar.activation(e_sb[:], b_sb[:], EXP)
                se = wpool.tile([PT, K], F32, tag=f"se{sfx}", name="se")
                nc.vector.tensor_reduce(
                    se[:], _ap(e_sb, [[N, K], [1, N]]), AX, ADD)
                r = wpool.tile([PT, K], F32, tag=f"r{sfx}", name="r")
                nc.vector.reciprocal(r[:], se[:])
                c_h = wpool.tile([PT, KN], F16, tag=f"c{sfx}", name="c_h")
                nc.vector.tensor_tensor(
                    c_h[:], e_sb[:],
                    _ap(r, [[1, K], [0, N]]),
                    MULT)

                # ---- s[l,(k,j)] = sum_n c*p2, p2 in (k,j,n) layout:
                # c broadcast over j (middle), n packed inner -> 2x ----
                tw = bigpool.tile([PT, KJN], F16, tag="tw", name="tw")
                nc.vector.tensor_tensor(
                    tw[:], p2[:],
                    _ap(c_h, [[N, K], [0, J], [1, N]]),
                    MULT)
                u1 = tpool.tile([PT, KJN // 2], F16, tag="u1", name="u1")
                nc.vector.tensor_tensor(
                    _ap(u1, [[16, KJ], [1, 16]]),
                    _ap(tw, [[N, KJ], [1, 16]]),
                    _apo(tw, 16, [[N, KJ], [1, 16]]),
                    ADD)
                u2 = tpool.tile([PT, KJN // 4], F16, tag="u2", name="u2")
                nc.vector.tensor_tensor(
                    _ap(u2, [[8, KJ], [1, 8]]),
                    _ap(u1, [[16, KJ], [1, 8]]),
                    _apo(u1, 8, [[16, KJ], [1, 8]]),
                    ADD)
                u3 = tpool.tile([PT, KJN // 8], F16, tag="u3", name="u3")
                nc.vector.tensor_tensor(
                    _ap(u3, [[4, KJ], [1, 4]]),
                    _ap(u2, [[8, KJ], [1, 4]]),
                    _apo(u2, 4, [[8, KJ], [1, 4]]),
                    ADD)
                u4 = tpool.tile([PT, KJN // 16], F16, tag="u4", name="u4")
                nc.vector.tensor_tensor(
                    _ap(u4, [[2, KJ], [1, 2]]),
                    _ap(u3, [[4, KJ], [1, 2]]),
                    _apo(u3, 2, [[4, KJ], [1, 2]]),
                    ADD)
                s2 = wpool.tile([PT, KJ], F32, tag="s2", name="s2")
                nc.vector.tensor_tensor(
                    s2[:],
                    _ap(u4, [[2, KJ]]),
                    _apo(u4, 1, [[2, KJ]]),
                    ADD)
                out_h = wpool.tile([PT, KJ], F16, tag=f"o{it}{sfx}",
                                   name="out_it")
                squash(s2, out_h, f"i{it}{sfx}")
                st["out"] = out_h

            # ---- schedule: tiles in pairs (A,B) with iteration-level
            # interleave; each tile's g-phase (PE/scalar) hides under the
            # other tile's DVE phase. Prefetch next pair's a1/a2 early. ----
            sts = {}
            groups = [[0, 1], [2, 3], [4, 5, 6]]
            for t in groups[0]:
                sts[t] = stage_a1x(t, t % 4)
            nc.sync.dma_start(wsum0[:], wsum_d[0])
            nc.sync.dma_start(wsum1[:], wsum_d[1])
            warm(wsum0, wsum1)
            for t in groups[0]:
                stage_a2(sts[t])
            nc.sync.dma_start(wg0[:], wg_d[0])
            nc.sync.dma_start(wg1[:], wg_d[1])
            nc.sync.dma_start(wbd0[:], wbd_d[0])
            nc.sync.dma_start(wbd1[:], wbd_d[1])
            warm(wg0, wg1)
            for t in groups[0]:
                stage_g(sts[t], 0)
            warm(wbd0, wbd1)
            stage_a1p(sts[0])
            for gi, grp in enumerate(groups):
                nxt = groups[gi + 1] if gi + 1 < len(groups) else []
                fillers = []
                if gi == 0:
                    fillers.append(lambda st=sts[1]: stage_a1p(st))
                def mk_first(t):
                    def f():
                        sts[t] = stage_a1x(t, t % 4)
                        stage_a1p(sts[t], 0, 8)
                    return f

                def mk_second(t):
                    return lambda: stage_a1p(sts[t], 8, 16)

                for t in nxt:
                    fillers.append(mk_first(t))
                    fillers.append(mk_second(t))
                for i, t in enumerate(grp):
                    stage_v(sts[t], 0)
                    if i >= 1:
                        stage_g(sts[grp[i - 1]], 1)
                    if fillers:
                        fillers.pop(0)()
                stage_g(sts[grp[-1]], 1)
                for i, t in enumerate(grp):
                    stage_v(sts[t], 1)
                    if fillers:
                        fillers.pop(0)()
                    if i < len(nxt):
                        stage_a2(sts[nxt[i]])
                if nxt:
                    stage_g(sts[nxt[0]], 0)
                for f in fillers:
                    f()
                if len(nxt) > len(grp):
                    stage_a2(sts[nxt[len(grp)]])
                for t in grp:
                    nc.sync.dma_start(out_d[t * PT:(t + 1) * PT, :],
                                      sts[t]["out"][:])
                for t in nxt[1:]:
                    stage_g(sts[t], 0)
                for t in grp:
                    del sts[t]
    nc.compile()
    return nc


def host_prep(inputs, W):
    x = np.ascontiguousarray(inputs, np.float32).reshape(NCORES, L, NI)
    x = x.astype(np.float16)
    # wsum[(n,i),(k,j)] for the uniform first iteration, 1/N folded in
    wsum = np.ascontiguousarray(
        W.transpose(1, 2, 0, 3).reshape(NI, KJ), np.float32) / N
    wsum = wsum.astype(np.float16).reshape(2, 128, KJ)
    # wbd: rows (n,i); cols in 512-chunks covering n-pairs, within
    # chunk (k,j,d) so psum chunk ch = p2[(k,j,2ch+d)], dst-inner [1,2].
    wbd_full = np.zeros((NI, KJN), np.float16)
    Wt = W.astype(np.float32)
    for ch in range(16):
        for d in range(2):
            n = 2 * ch + d
            blk = Wt[:, n].transpose(1, 0, 2).reshape(I, KJ)  # (i),(k,j)
            for k in range(K):
                for j in range(J):
                    wbd_full[n * I:(n + 1) * I,
                             ch * 512 + k * 32 + j * 2 + d] = blk[:, k * J + j]
    wbd = np.stack([wbd_full[0:128, 0:KJN // 2],
                    wbd_full[128:256, KJN // 2:]]).astype(np.float16)
    # wg: rows (k,j) (k in octave h), cols (k,n,i) restricted to octave:
    # wg[h][(k-8h)*16+j, (k-8h)*256 + n*8 + i] = W[k,n,i,j]
    wg = np.zeros((2, 128, KNI // 2), np.float16)
    for h in range(2):
        for k8 in range(8):
            k = 8 * h + k8
            # (j) x (n,i)
            blk = Wt[k].transpose(2, 0, 1).reshape(J, NI)
            wg[h, k8 * 16:(k8 + 1) * 16,
               k8 * 256:(k8 + 1) * 256] = blk.astype(np.float16)
    eye = np.eye(128, dtype=np.float16)
    return x, wsum, wbd, wg, eye


_CACHED = {}


def kernel(inputs, W):
    from concourse.bass_utils import run_bass_kernel_spmd

    x, wsum, wbd, wg, eye = host_prep(inputs, W)
    if "nc" not in _CACHED:
        _CACHED["nc"] = build_bass()
    nc = _CACHED["nc"]
    in_maps = [{"x": np.ascontiguousarray(x[c]), "wsum": wsum, "wbd": wbd,
                "wg": wg, "eye": eye} for c in range(NCORES)]
    res = run_bass_kernel_spmd(nc, in_maps, core_ids=list(range(NCORES)))
    out = np.stack([res.results[c]["out"] for c in range(NCORES)])
    return out.astype(np.float32).reshape(B, R, C, K, J)


# revision 27
# speedup vs baseline: 1.0104x; 1.0104x over previous
"""CapsLayer2D dynamic-routing kernel for 8 Trainium2 NeuronCores.

Full inputs:  inputs [32,14,14,32,8] f32, W [16,32,8,16] f32
Full output:  out [32,14,14,16,16] f32

Sharding: pure data parallel over batch (4 batches / core -> 784 routing
locations per core). W replicated.

v4: p2 stored in (k,j,n) layout so the weighted-sum multiply reads c
directly through a middle-broadcast AP at DVE 2x rate (no scalar cj
expansion); the agreement p2.out contraction is refactored through the
tensor engine as g[l,k,n,i] = sum_j W[k,n,i,j] out[l,k,j] (matmuls
against transposed out), leaving only a 4096-wide multiply + i-tree on
the DVE; reduction trees keep the reduced dim innermost so every stage
runs packed fp16 at 2x; output is DMA'd fp16 and widened on host.
Tiles are processed in pairs with iteration-level interleave so each
tile's PE/scalar g-phase hides under the other tile's DVE phase.
"""

import sys

sys.path.insert(0, "/opt/trn_rl_repo")

import numpy as np

import concourse.bass as bass
import concourse.mybir as mybir
from concourse.bacc import Bacc
from concourse.tile import TileContext

F32 = mybir.dt.float32
F16 = mybir.dt.float16
ADD = mybir.AluOpType.add
MULT = mybir.AluOpType.mult
DIV = mybir.AluOpType.divide
AX = mybir.AxisListType.X
EXP = mybir.ActivationFunctionType.Exp
SQRT = mybir.ActivationFunctionType.Sqrt
SQUARE = mybir.ActivationFunctionType.Square

EPS = 1e-7
B, R, C, N, I = 32, 14, 14, 32, 8
K, J = 16, 16
NCORES = 8
BC = B // NCORES            # batches per core
L = BC * R * C              # 784 locations per core
PT = 112                    # locations per partition-tile
NT = L // PT                # 7 tiles
NI = N * I                  # 256
KJ = K * J                  # 256
JN = J * N                  # 512 (= k-stride in p2 (k,j,n))
KN = K * N                  # 512
KNI = K * N * I             # 4096
KJN = K * J * N             # 8192


def _ap(base, dims):
    """AP over tile `base` ([part, free] contiguous) with free dims
    [(step,count)...] in elements; step 0 = broadcast."""
    return bass.AP(base.tensor, base.offset, [list(base.ap[0])] + [list(d) for d in dims])


def _apo(base, off, dims):
    """Same, with an element offset into the free dim."""
    return bass.AP(base.tensor, base.offset + off,
                   [list(base.ap[0])] + [list(d) for d in dims])


def build_bass():
    nc = Bacc()
    x_d = nc.declare_dram_parameter("x", [L, NI], F16, isOutput=False)
    wsum_d = nc.declare_dram_parameter("wsum", [2, 128, KJ], F16, isOutput=False)
    wbd_d = nc.declare_dram_parameter("wbd", [2, 128, KJN // 2], F16, isOutput=False)
    wg_d = nc.declare_dram_parameter("wg", [2, 128, KNI // 2], F16, isOutput=False)
    eye_d = nc.declare_dram_parameter("eye", [128, 128], F16, isOutput=False)
    out_d = nc.declare_dram_parameter("out", [L, KJ], F16, isOutput=True)

    with TileContext(nc) as tc:
        import contextlib
        ctx = contextlib.ExitStack()
        with ctx:
            cpool = ctx.enter_context(tc.tile_pool(name="const", bufs=1))
            wpool = ctx.enter_context(tc.tile_pool(name="work", bufs=1))
            bigpool = ctx.enter_context(tc.tile_pool(name="big", bufs=1))
            tpool = ctx.enter_context(tc.tile_pool(name="tree", bufs=1))
            pspool = ctx.enter_context(tc.tile_pool(name="ps", bufs=1, space="PSUM"))
            psmm = ctx.enter_context(tc.tile_pool(name="psmm", bufs=2, space="PSUM"))
            psg = ctx.enter_context(tc.tile_pool(name="psg", bufs=2, space="PSUM"))

            wsum0 = cpool.tile([128, KJ], F16)
            wsum1 = cpool.tile([128, KJ], F16)
            wbd0 = cpool.tile([128, KJN // 2], F16)
            wbd1 = cpool.tile([128, KJN // 2], F16)
            wg0 = cpool.tile([128, KNI // 2], F16)
            wg1 = cpool.tile([128, KNI // 2], F16)
            eye = cpool.tile([128, 128], F16)
            nc.sync.dma_start(eye[:], eye_d[:])

            # PE warm-up: absorb each const DMA tick into PE's vector clock
            # one at a time, so no later LDWEIGHTS needs >1 sync wait
            # (HW limit: one wait slot on LDW).
            def warm(*wts):
                for wt in wts:
                    ps_w = psmm.tile([112, 512], F32, tag="mm", name="ps_w")
                    nc.tensor.matmul(ps_w[:, :KJ], wt[:, :112], wt[:, :KJ],
                                     start=True, stop=True)
            ps_wt = pspool.tile([128, 128], F16, tag="psT", name="ps_wt")
            nc.tensor.transpose(ps_wt[:], eye[:], eye[:])

            def squash(s_sb, out_sb, tag):
                """out = squash(s) over j; s_sb [PT,KJ] f32, out_sb [PT,KJ]
                f16 ((k,j) layout). f = sq/((1+sq)*sqrt(sq+eps)); out = s*f."""
                ssq = wpool.tile([PT, KJ], F16, tag=f"ssq{tag}", name=f"ssq{tag}")
                sq = wpool.tile([PT, K], F32, tag=f"sq{tag}", name=f"sq{tag}")
                rti = wpool.tile([PT, K], F32, tag=f"rti{tag}", name=f"rti{tag}")
                rt = wpool.tile([PT, K], F32, tag=f"rt{tag}", name=f"rt{tag}")
                den = wpool.tile([PT, K], F32, tag=f"den{tag}", name=f"den{tag}")
                rden = wpool.tile([PT, K], F32, tag=f"rd{tag}", name=f"rd{tag}")
                f = wpool.tile([PT, K], F32, tag=f"f{tag}", name=f"f{tag}")
                nc.scalar.activation(ssq[:], s_sb[:], SQUARE)
                nc.vector.tensor_reduce(
                    sq[:], _ap(ssq, [[J, K], [1, J]]), AX, ADD)
                nc.vector.tensor_scalar(rti[:], sq[:], EPS, None, ADD)
                nc.scalar.activation(rt[:], rti[:], SQRT)
                # den = (sq + 1) * rt, fused on DVE
                nc.vector.scalar_tensor_tensor(
                    den[:], sq[:], 1.0, rt[:], ADD, MULT)
                nc.vector.reciprocal(rden[:], den[:])
                nc.vector.tensor_tensor(f[:], sq[:], rden[:], MULT)
                nc.vector.tensor_tensor(
                    _ap(out_sb, [[J, K], [1, J]]),
                    _ap(s_sb, [[J, K], [1, J]]),
                    _ap(f, [[1, K], [0, J]]),
                    MULT)

            def stage_a1x(t, sfx):
                """x DMA + transposes. Returns state dict."""
                st = {"t": t, "sfx": sfx}
                x_sb = wpool.tile([PT, NI], F16, tag=f"x{sfx}", name="x_sb")
                nc.sync.dma_start(x_sb[:], x_d[t * PT:(t + 1) * PT, :])
                st["x"] = x_sb
                xt = []
                for h in range(2):
                    ps_t = pspool.tile([128, PT], F16, tag="psT", name="ps_t")
                    xth = wpool.tile([128, PT], F16, tag=f"xT{h}{sfx}",
                                     name=f"xT{h}")
                    nc.tensor.transpose(
                        ps_t[:], x_sb[:, h * 128:(h + 1) * 128], eye[:PT, :PT])
                    nc.scalar.copy(xth[:], ps_t[:])
                    xt.append(xth)
                st["xt"] = xt
                return st

            def stage_a1p(st, lo=0, hi=16):
                """p2 matmuls + psum->sbuf copies; p2 layout (k,j,n).
                chunk ch covers n-pair (2ch,2ch+1); within-chunk psum cols
                (k,j,d) so the copy is dst-inner [1,2]."""
                sfx = st["sfx"]
                if lo == 0:
                    p2 = bigpool.tile([PT, KJN], F16, tag=f"p2{sfx}",
                                      name="p2")
                    st["p2"] = p2
                p2 = st["p2"]
                for ch in range(lo, hi):
                    h = ch // 8
                    wb = (wbd0, wbd1)[h]
                    ps = psmm.tile([PT, 512], F32, tag="mm", name="ps_mm")
                    nc.tensor.matmul(
                        ps[:], st["xt"][h][:],
                        wb[:, (ch % 8) * 512:(ch % 8 + 1) * 512],
                        start=True, stop=True)
                    dst = _apo(p2, 2 * ch, [[JN, K], [N, J], [1, 2]])
                    src = _ap(ps, [[32, K], [2, J], [1, 2]])
                    nc.scalar.copy(dst, src)

            def stage_a2(st):
                """Iteration 1 (uniform c): s = mean_n p2, squash."""
                xt = st["xt"]
                sfx = st["sfx"]
                ps_s = pspool.tile([PT, KJ], F32, tag="s", name="ps_s")
                nc.tensor.matmul(ps_s[:], xt[0][:], wsum0[:], start=True, stop=False)
                nc.tensor.matmul(ps_s[:], xt[1][:], wsum1[:], start=False, stop=True)
                out_h = wpool.tile([PT, KJ], F16, tag=f"oh{sfx}", name="out_h")
                squash(ps_s, out_h, f"a{sfx}")
                st["out"] = out_h

            def stage_g(st, it):
                """PE/scalar phase: outT transposes + g matmuls + copies.
                g[l,(k,n,i)] = sum_j W[k,n,i,j] * out[l,k,j], fp16 SBUF."""
                sfx = st["sfx"]
                out_h = st["out"]
                g_sb = tpool.tile([PT, KNI], F16, tag=f"g{sfx % 3}",
                                  name="g_sb")
                for h in range(2):
                    ps_t = pspool.tile([128, PT], F16, tag="psT", name="ps_ot")
                    oth = wpool.tile([128, PT], F16, tag=f"oT{h}{sfx}",
                                     name=f"oT{h}")
                    nc.tensor.transpose(
                        ps_t[:], out_h[:, h * 128:(h + 1) * 128],
                        eye[:PT, :PT])
                    nc.scalar.copy(oth[:], ps_t[:])
                    wgh = (wg0, wg1)[h]
                    for piece in range(4):
                        g_ps = psg.tile([PT, 512], F32, tag="gps",
                                        name="g_ps")
                        nc.tensor.matmul(
                            g_ps[:], oth[:, :PT],
                            wgh[:, piece * 512:(piece + 1) * 512],
                            start=True, stop=True)
                        nc.scalar.copy(
                            g_sb[:, (h * 4 + piece) * 512:
                                 (h * 4 + piece + 1) * 512], g_ps[:])
                st["g"] = g_sb

            def stage_v(st, it):
                """DVE phase: agreement (x*g + i-tree) -> b; softmax -> c;
                weighted sum (p2*c_bcast + n-tree) -> s2; squash -> out."""
                sfx = st["sfx"]
                p2, g_sb, x_sb = st["p2"], st["g"], st["x"]
                # ---- agreement: tmp_a[l,(k,n,i)] = x[l,(n,i)] * g ----
                tmp_a = tpool.tile([PT, KNI], F16, tag="ta", name="tmp_a")
                nc.vector.tensor_tensor(
                    tmp_a[:], g_sb[:],
                    _ap(x_sb, [[0, K], [I, N], [1, I]]),
                    MULT)
                a1 = tpool.tile([PT, KNI // 2], F16, tag="a1", name="a1")
                nc.vector.tensor_tensor(
                    _ap(a1, [[4, KN], [1, 4]]),
                    _ap(tmp_a, [[8, KN], [1, 4]]),
                    _apo(tmp_a, 4, [[8, KN], [1, 4]]),
                    ADD)
                a2 = tpool.tile([PT, KNI // 4], F16, tag="a2", name="a2")
                nc.vector.tensor_tensor(
                    _ap(a2, [[2, KN], [1, 2]]),
                    _ap(a1, [[4, KN], [1, 2]]),
                    _apo(a1, 2, [[4, KN], [1, 2]]),
                    ADD)
                if it == 0:
                    b_sb = wpool.tile([PT, KN], F32, tag=f"b{sfx}", name="b_sb")
                    nc.vector.tensor_tensor(
                        b_sb[:],
                        _ap(a2, [[2, KN]]),
                        _apo(a2, 1, [[2, KN]]),
                        ADD)
                    st["b"] = b_sb
                else:
                    a3 = tpool.tile([PT, KNI // 8], F16, tag="a3",
                                    name="a3")
                    nc.vector.tensor_tensor(
                        a3[:],
                        _ap(a2, [[2, KN]]),
                        _apo(a2, 1, [[2, KN]]),
                        ADD)
                    b_sb = st["b"]
                    nc.vector.tensor_tensor(b_sb[:], b_sb[:], a3[:], ADD)

                # ---- softmax over n (b bounded; no max-sub), c = e*r ----
                e_sb = wpool.tile([PT, KN], F32, tag=f"e{sfx}", name="e_sb")
                nc.scalar.activation(e_sb[:], b_sb[:], EXP)
                se = wpool.tile([PT, K], F32, tag=f"se{sfx}", name="se")
                nc.vector.tensor_reduce(
                    se[:], _ap(e_sb, [[N, K], [1, N]]), AX, ADD)
                r = wpool.tile([PT, K], F32, tag=f"r{sfx}", name="r")
                nc.vector.reciprocal(r[:], se[:])
                c_h = wpool.tile([PT, KN], F16, tag=f"c{sfx}", name="c_h")
                nc.vector.tensor_tensor(
                    c_h[:], e_sb[:],
                    _ap(r, [[1, K], [0, N]]),
                    MULT)

                # ---- s[l,(k,j)] = sum_n c*p2, p2 in (k,j,n) layout:
                # c broadcast over j (middle), n packed inner -> 2x ----
                tw = bigpool.tile([PT, KJN], F16, tag="tw", name="tw")
                nc.vector.tensor_tensor(
                    tw[:], p2[:],
                    _ap(c_h, [[N, K], [0, J], [1, N]]),
                    MULT)
                u1 = tpool.tile([PT, KJN // 2], F16, tag="u1", name="u1")
                nc.vector.tensor_tensor(
                    _ap(u1, [[16, KJ], [1, 16]]),
                    _ap(tw, [[N, KJ], [1, 16]]),
                    _apo(tw, 16, [[N, KJ], [1, 16]]),
                    ADD)
                u2 = tpool.tile([PT, KJN // 4], F16, tag="u2", name="u2")
                nc.vector.tensor_tensor(
                    _ap(u2, [[8, KJ], [1, 8]]),
                    _ap(u1, [[16, KJ], [1, 8]]),
                    _apo(u1, 8, [[16, KJ], [1, 8]]),
                    ADD)
                u3 = tpool.tile([PT, KJN // 8], F16, tag="u3", name="u3")
                nc.vector.tensor_tensor(
                    _ap(u3, [[4, KJ], [1, 4]]),
                    _ap(u2, [[8, KJ], [1, 4]]),
                    _apo(u2, 4, [[8, KJ], [1, 4]]),
                    ADD)
                u4 = tpool.tile([PT, KJN // 16], F16, tag="u4", name="u4")
                nc.vector.tensor_tensor(
                    _ap(u4, [[2, KJ], [1, 2]]),
                    _ap(u3, [[4, KJ], [1, 2]]),
                    _apo(u3, 2, [[4, KJ], [1, 2]]),
                    ADD)
                s2 = wpool.tile([PT, KJ], F32, tag="s2", name="s2")
                nc.vector.tensor_tensor(
                    s2[:],
                    _ap(u4, [[2, KJ]]),
                    _apo(u4, 1, [[2, KJ]]),
                    ADD)
                out_h = wpool.tile([PT, KJ], F16, tag=f"o{it}{sfx}",
                                   name="out_it")
                squash(s2, out_h, f"i{it}{sfx}")
                st["out"] = out_h

            # ---- schedule: tiles in pairs (A,B) with iteration-level
            # interleave; each tile's g-phase (PE/scalar) hides under the
            # other tile's DVE phase. Prefetch next pair's a1/a2 early. ----
            sts = {}
            groups = [[0, 1], [2, 3], [4, 5, 6]]
            for t in groups[0]:
                sts[t] = stage_a1x(t, t % 4)
            nc.sync.dma_start(wsum0[:], wsum_d[0])
            nc.sync.dma_start(wsum1[:], wsum_d[1])
            warm(wsum0, wsum1)
            for t in groups[0]:
                stage_a2(sts[t])
            nc.sync.dma_start(wg0[:], wg_d[0])
            nc.sync.dma_start(wg1[:], wg_d[1])
            nc.sync.dma_start(wbd0[:], wbd_d[0])
            nc.sync.dma_start(wbd1[:], wbd_d[1])
            warm(wg0, wg1)
            for t in groups[0]:
                stage_g(sts[t], 0)
            warm(wbd0, wbd1)
            stage_a1p(sts[0])
            for gi, grp in enumerate(groups):
                nxt = groups[gi + 1] if gi + 1 < len(groups) else []
                fillers = []
                if gi == 0:
                    fillers.append(lambda st=sts[1]: stage_a1p(st))
                def mk_first(t):
                    def f():
                        sts[t] = stage_a1x(t, t % 4)
                        stage_a1p(sts[t], 0, 8)
                    return f

                def mk_second(t):
                    return lambda: stage_a1p(sts[t], 8, 16)

                for t in nxt:
                    fillers.append(mk_first(t))
                    fillers.append(mk_second(t))
                for t in grp:
                    stage_v(sts[t], 0)
                    if fillers:
                        fillers.pop(0)()
                for t in grp:
                    stage_g(sts[t], 1)
                for i, t in enumerate(grp):
                    stage_v(sts[t], 1)
                    if fillers:
                        fillers.pop(0)()
                    if i < len(nxt):
                        stage_a2(sts[nxt[i]])
                if nxt:
                    stage_g(sts[nxt[0]], 0)
                for f in fillers:
                    f()
                if len(nxt) > len(grp):
                    stage_a2(sts[nxt[len(grp)]])
                for t in grp:
                    nc.sync.dma_start(out_d[t * PT:(t + 1) * PT, :],
                                      sts[t]["out"][:])
                for t in nxt[1:]:
                    stage_g(sts[t], 0)
                for t in grp:
                    del sts[t]
    nc.compile()
    return nc


def host_prep(inputs, W):
    x = np.ascontiguousarray(inputs, np.float32).reshape(NCORES, L, NI)
    x = x.astype(np.float16)
    # wsum[(n,i),(k,j)] for the uniform first iteration, 1/N folded in
    wsum = np.ascontiguousarray(
        W.transpose(1, 2, 0, 3).reshape(NI, KJ), np.float32) / N
    wsum = wsum.astype(np.float16).reshape(2, 128, KJ)
    # wbd: rows (n,i); cols in 512-chunks covering n-pairs, within
    # chunk (k,j,d) so psum chunk ch = p2[(k,j,2ch+d)], dst-inner [1,2].
    wbd_full = np.zeros((NI, KJN), np.float16)
    Wt = W.astype(np.float32)
    for ch in range(16):
        for d in range(2):
            n = 2 * ch + d
            blk = Wt[:, n].transpose(1, 0, 2).reshape(I, KJ)  # (i),(k,j)
            for k in range(K):
                for j in range(J):
                    wbd_full[n * I:(n + 1) * I,
                             ch * 512 + k * 32 + j * 2 + d] = blk[:, k * J + j]
    wbd = np.stack([wbd_full[0:128, 0:KJN // 2],
                    wbd_full[128:256, KJN // 2:]]).astype(np.float16)
    # wg: rows (k,j) (k in octave h), cols (k,n,i) restricted to octave:
    # wg[h][(k-8h)*16+j, (k-8h)*256 + n*8 + i] = W[k,n,i,j]
    wg = np.zeros((2, 128, KNI // 2), np.float16)
    for h in range(2):
        for k8 in range(8):
            k = 8 * h + k8
            # (j) x (n,i)
            blk = Wt[k].transpose(2, 0, 1).reshape(J, NI)
            wg[h, k8 * 16:(k8 + 1) * 16,
               k8 * 256:(k8 + 1) * 256] = blk.astype(np.float16)
    eye = np.eye(128, dtype=np.float16)
    return x, wsum, wbd, wg, eye


_CACHED = {}


def kernel(inputs, W):
    from concourse.bass_utils import run_bass_kernel_spmd

    x, wsum, wbd, wg, eye = host_prep(inputs, W)
    if "nc" not in _CACHED:
        _CACHED["nc"] = build_bass()
    nc = _CACHED["nc"]
    in_maps = [{"x": np.ascontiguousarray(x[c]), "wsum": wsum, "wbd": wbd,
                "wg": wg, "eye": eye} for c in range(NCORES)]
    res = run_bass_kernel_spmd(nc, in_maps, core_ids=list(range(NCORES)))
    out = np.stack([res.results[c]["out"] for c in range(NCORES)])
    return out.astype(np.float32).reshape(B, R, C, K, J)
